# revision 12
# baseline (speedup 1.0000x reference)
"""Joint bilateral filter (3x3, reflect pad) on 8 trn2 cores.

Sharding: 1024 output rows (2 batches x 512 H) split as 8 x 128 rows.
Host pre-pads H and W with reflect (radius 1), converts to fp16, and
packs column-chunk slabs so every DMA is one long contiguous run per
partition.

Device pipeline per column chunk (96/64 output cols):
  - src loaded from HBM once (center dy=1 tile); the dy=0/2 row-shifted
    copies are built by partition-shifted SBUF->SBUF DMA plus a 1-row
    halo load, halving src HBM traffic vs. three shifted loads.
  - center tap weight is exactly 1 (exp(0)*w1(0,0)), so only the 8
    off-center taps need weights/multiplies; the center rides the PE
    pass unscaled and contributes the constant 1 to the denominator.
  - weights: per-tap subs on DVE (fp16, 2x mode), per-dy batched squares
    and exps on Act; the spatial ln(w1) constant folds into the weight
    argument via a fused scalar_tensor_tensor on Pool, so exp needs no
    per-tap bias and batches across dx.
  - per-tap products split DVE/Pool, fp16.
  - the 9-tap accumulation runs on the otherwise idle PE as identity
    matmuls accumulating in PSUM (fp32), one single-bank [128,21,<=24]
    subslice per accumulation group.
  - normalization (num * 1/den) on Pool straight out of PSUM.
"""

import sys

sys.path.insert(0, "/opt/trn_rl_repo")

import numpy as np

B, H, W = 2, 512, 512
CS, CI = 21, 3
N_CORES = 8
ROWS = (B * H) // N_CORES  # 128 output rows per core
CHUNKS = [(0, 96), (96, 96), (192, 96), (288, 96), (384, 64), (448, 64)]
NCH = len(CHUNKS)
SLOT = 98  # slab width per chunk (96+2); 64-col chunks use 66 of it
OSLOT = 96
SUB = 24  # psum subslice output columns (21*24*4B = 2016B = 1 bank)

INV2SIG2 = 8.0  # 1/(2*0.25^2)

# off-center taps (dy, dx), grouped by dy; wk9 slot = list index
TAPS = [(0, 0), (0, 1), (0, 2), (1, 0), (1, 2), (2, 0), (2, 1), (2, 2)]
DY_SLOTS = {0: (0, 3), 1: (3, 5), 2: (5, 8)}  # dy -> wk9 slot range

_CACHE = {}


def _sub_ranges(wc):
    r = []
    x = 0
    while x < wc:
        r.append((x, min(SUB, wc - x)))
        x += SUB
    return r


def _build():
    from concourse.bacc import Bacc
    from concourse.tile import TileContext
    from concourse.masks import make_identity
    import concourse.mybir as mybir

    fp32 = mybir.dt.float32
    fp16 = mybir.dt.float16
    Alu = mybir.AluOpType
    Act = mybir.ActivationFunctionType

    nc = Bacc("TRN2", target_bir_lowering=False, debug=False, num_devices=N_CORES)
    src_d = nc.dram_tensor("src", [ROWS + 2, NCH, CS, SLOT], fp16, kind="ExternalInput")
    im_d = nc.dram_tensor("im", [ROWS + 2, NCH, CI, SLOT], fp16, kind="ExternalInput")
    out_d = nc.dram_tensor("out", [ROWS, NCH, CS, OSLOT], fp16, kind="ExternalOutput")

    with TileContext(nc) as tc:
        with tc.tile_pool(name="p", bufs=1) as pool, tc.psum_pool(
            name="ps", bufs=1
        ) as psp:
            ident = pool.tile([ROWS, ROWS], fp16, tag="ident")
            make_identity(nc, ident[:])

            for ci, (x0, wc) in enumerate(CHUNKS):
                # --- loads: src center from HBM, dy 0/2 via shifted SBUF copy
                s1 = pool.tile([ROWS, CS, SLOT], fp16, tag="s1", bufs=2)
                nc.sync.dma_start(
                    s1[:].rearrange("p c w -> p (c w)"),
                    src_d[1 : 1 + ROWS, ci].rearrange("p c w -> p (c w)"),
                )
                s0 = pool.tile([ROWS, CS, SLOT], fp16, tag="s0", bufs=2)
                nc.scalar.dma_start(
                    s0[1:ROWS].rearrange("p c w -> p (c w)"),
                    s1[0 : ROWS - 1].rearrange("p c w -> p (c w)"),
                )
                nc.sync.dma_start(
                    s0[0:1].rearrange("p c w -> p (c w)"),
                    src_d[0:1, ci].rearrange("p c w -> p (c w)"),
                )
                s2 = pool.tile([ROWS, CS, SLOT], fp16, tag="s2", bufs=2)
                nc.scalar.dma_start(
                    s2[0 : ROWS - 1].rearrange("p c w -> p (c w)"),
                    s1[1:ROWS].rearrange("p c w -> p (c w)"),
                )
                nc.sync.dma_start(
                    s2[ROWS - 1 : ROWS].rearrange("p c w -> p (c w)"),
                    src_d[ROWS + 1 : ROWS + 2, ci].rearrange("p c w -> p (c w)"),
                )
                s_t = [s0, s1, s2]
                i_t = []
                for dy in range(3):
                    it = pool.tile([ROWS, CI, SLOT], fp16, tag=f"i{dy}", bufs=2)
                    nc.sync.dma_start(
                        it[:].rearrange("p c w -> p (c w)"),
                        im_d[dy : dy + ROWS, ci].rearrange("p c w -> p (c w)"),
                    )
                    i_t.append(it)
                ic = i_t[1][:, :, 1 : 1 + wc]

                # --- bilateral weights for the 8 off-center taps
                wk9 = pool.tile([ROWS, 8, OSLOT], fp16, tag="wk9", bufs=2)
                d3 = {
                    dy: pool.tile(
                        [ROWS, DY_SLOTS[dy][1] - DY_SLOTS[dy][0], CI, OSLOT],
                        fp16,
                        tag=f"d3_{dy}",
                        name=f"d3_{dy}",
                        bufs=2,
                    )
                    for dy in range(3)
                }
                d23 = {
                    dy: pool.tile(
                        [ROWS, DY_SLOTS[dy][1] - DY_SLOTS[dy][0], CI, OSLOT],
                        fp32,
                        tag=f"d23_{dy}",
                        name=f"d23_{dy}",
                        bufs=2,
                    )
                    for dy in range(3)
                }
                wr = {
                    dy: pool.tile(
                        [ROWS, DY_SLOTS[dy][1] - DY_SLOTS[dy][0], OSLOT],
                        fp32,
                        tag=f"wr_{dy}",
                        name=f"wr_{dy}",
                        bufs=2,
                    )
                    for dy in range(3)
                }
                # subs per tap (DVE), into the per-dy batch tile
                for slot, (dy, dx) in enumerate(TAPS):
                    j = slot - DY_SLOTS[dy][0]
                    nc.vector.tensor_tensor(
                        d3[dy][:, j, :, :wc],
                        i_t[dy][:, :, dx : dx + wc],
                        ic,
                        Alu.subtract,
                    )
                # squares per dy (Act), fp32 out
                for dy in range(3):
                    nc.scalar.square(
                        d23[dy][:, :, :, :wc], d3[dy][:, :, :, :wc]
                    )
                # wr = d2_c0 + d2_c1 (per-dy batch, Pool), then per-tap fused
                # (+spatial const, +d2_c2) via scalar_tensor_tensor (Pool)
                for dy in range(3):
                    nc.gpsimd.tensor_tensor(
                        wr[dy][:, :, :wc],
                        d23[dy][:, :, 0, :wc],
                        d23[dy][:, :, 1, :wc],
                        Alu.add,
                    )
                for slot, (dy, dx) in enumerate(TAPS):
                    j = slot - DY_SLOTS[dy][0]
                    ck = ((dx - 1) ** 2 + (dy - 1) ** 2) / (2.0 * INV2SIG2)
                    nc.vector.scalar_tensor_tensor(
                        wr[dy][:, j, :wc],
                        wr[dy][:, j, :wc],
                        ck,
                        d23[dy][:, j, 2, :wc],
                        Alu.add,
                        Alu.add,
                    )
                # exp per dy (Act), no bias needed
                for dy in range(3):
                    lo, hi = DY_SLOTS[dy]
                    nc.scalar.activation(
                        wk9[:, lo:hi, :wc], wr[dy][:, :, :wc], Act.Exp,
                        scale=-INV2SIG2,
                    )

                # --- den = 1 + sum(wk), reciprocal
                den = pool.tile([ROWS, OSLOT], fp16, tag="den", bufs=2)
                rd = pool.tile([ROWS, OSLOT], fp16, tag="rd", bufs=2)
                nc.gpsimd.tensor_scalar(
                    den[:, :wc], wk9[:, 0, :wc], 1.0, None, Alu.add
                )
                for slot in range(1, 8):
                    nc.gpsimd.tensor_tensor(
                        den[:, :wc], den[:, :wc], wk9[:, slot, :wc], Alu.add
                    )
                with nc.allow_low_precision(reason="den in fp16 is plenty"):
                    nc.vector.reciprocal(rd[:, :wc], den[:, :wc])

                # --- products for the 8 off-center taps, split DVE/Pool
                prod = pool.tile([ROWS, 8, CS, OSLOT], fp16, tag="prod", bufs=2)
                CSPLIT = 9  # tap 4 split by channel between DVE and Pool
                for slot, (dy, dx) in enumerate(TAPS):
                    wk_b = (
                        wk9[:, slot, :wc]
                        .rearrange("p (o x) -> p o x", o=1)
                        .broadcast_to([ROWS, CS, wc])
                    )
                    sk = s_t[dy][:, :, dx : dx + wc]
                    if slot < 4:
                        nc.vector.tensor_tensor(
                            prod[:, slot, :, :wc], sk, wk_b, Alu.mult
                        )
                    elif slot > 4:
                        nc.gpsimd.tensor_tensor(
                            prod[:, slot, :, :wc], sk, wk_b, Alu.mult
                        )
                    else:
                        nc.vector.tensor_tensor(
                            prod[:, slot, :CSPLIT, :wc],
                            sk[:, :CSPLIT],
                            wk_b[:, :CSPLIT],
                            Alu.mult,
                        )
                        nc.gpsimd.tensor_tensor(
                            prod[:, slot, CSPLIT:, :wc],
                            sk[:, CSPLIT:],
                            wk_b[:, CSPLIT:],
                            Alu.mult,
                        )

                # --- 9-tap accumulation on the PE; Act drains PSUM to fp16
                nsb = pool.tile([ROWS, CS, OSLOT], fp16, tag="nsb", bufs=2)
                outt = pool.tile([ROWS, CS, OSLOT], fp16, tag="outt", bufs=2)
                for sx, sw in _sub_ranges(wc):
                    ps = psp.tile(
                        [ROWS, CS, sw], fp32, tag=f"ps{sw}", bufs=6 if sw == SUB else 2
                    )
                    nc.tensor.matmul(
                        ps[:].rearrange("p c x -> p (c x)"),
                        ident[:],
                        s1[:, :, 1 + sx : 1 + sx + sw],
                        start=True,
                        stop=False,
                    )
                    for slot in range(8):
                        nc.tensor.matmul(
                            ps[:].rearrange("p c x -> p (c x)"),
                            ident[:],
                            prod[:, slot, :, sx : sx + sw],
                            start=False,
                            stop=(slot == 7),
                        )
                    nc.scalar.copy(nsb[:, :, sx : sx + sw], ps[:])
                rd_b = (
                    rd[:, :wc]
                    .rearrange("p (o x) -> p o x", o=1)
                    .broadcast_to([ROWS, CS, wc])
                )
                nc.vector.tensor_tensor(
                    outt[:, :, :wc], nsb[:, :, :wc], rd_b, Alu.mult
                )
                nc.scalar.dma_start(out_d[:, ci, :, :wc], outt[:, :, :wc])
    nc.compile()
    return nc


def _get_nc():
    if "nc" not in _CACHE:
        _CACHE["nc"] = _build()
    return _CACHE["nc"]


def _shard_inputs(src, im):
    srcp = np.pad(src, ((0, 0), (1, 1), (1, 1), (0, 0)), mode="reflect")
    imp = np.pad(im, ((0, 0), (1, 1), (1, 1), (0, 0)), mode="reflect")
    # channel-major fp16: [B, Hp, C, Wp]
    srcp = np.transpose(srcp, (0, 1, 3, 2)).astype(np.float16)
    imp = np.transpose(imp, (0, 1, 3, 2)).astype(np.float16)
    Hp = H + 2
    src_sl = np.zeros((B, Hp, NCH, CS, SLOT), np.float16)
    im_sl = np.zeros((B, Hp, NCH, CI, SLOT), np.float16)
    for ci, (x0, wc) in enumerate(CHUNKS):
        src_sl[:, :, ci, :, : wc + 2] = srcp[:, :, :, x0 : x0 + wc + 2]
        im_sl[:, :, ci, :, : wc + 2] = imp[:, :, :, x0 : x0 + wc + 2]
    in_maps = []
    for core in range(N_CORES):
        b, r0 = core // (N_CORES // B), (core % (N_CORES // B)) * ROWS
        in_maps.append(
            {
                "src": np.ascontiguousarray(src_sl[b, r0 : r0 + ROWS + 2]),
                "im": np.ascontiguousarray(im_sl[b, r0 : r0 + ROWS + 2]),
            }
        )
    return in_maps


def _unshard_output(results):
    out = np.empty((B, H, W, CS), dtype=np.float32)
    for core in range(N_CORES):
        b, r0 = core // (N_CORES // B), (core % (N_CORES // B)) * ROWS
        o = results[core]["out"]  # [128, NCH, 21, 96] fp16
        o = np.asarray(o).reshape(ROWS, NCH, CS, OSLOT)
        for ci, (x0, wc) in enumerate(CHUNKS):
            out[b, r0 : r0 + ROWS, x0 : x0 + wc] = np.transpose(
                o[:, ci, :, :wc], (0, 2, 1)
            ).astype(np.float32)
    return out


def kernel(src, im, _trace=False, _tmpdir=None):
    from concourse import bass_utils

    src = np.asarray(src, dtype=np.float32)
    im = np.asarray(im, dtype=np.float32)
    nc = _get_nc()
    in_maps = _shard_inputs(src, im)
    res = bass_utils.run_bass_kernel_spmd(
        nc, in_maps, core_ids=list(range(N_CORES)), trace=_trace, tmpdir=_tmpdir
    )
    _CACHE["last_results"] = res
    return _unshard_output(res.results)


# revision 13
# speedup vs baseline: 2.3425x; 2.3425x over previous
"""Joint bilateral filter (3x3, reflect pad) on 8 trn2 cores.

Sharding: 1024 output rows (2 batches x 512 H) split as 8 x 128 rows.
Host pre-pads H and W with reflect (radius 1), converts to fp16, and
packs src+im together into per-column-chunk slabs so each of the three
dy-shifted windows is ONE long-contiguous-run HBM DMA (SBUF<->SBUF
partition-shift DMA measured ~7x slower than HBM loads on HW, so the
three windows are simply re-read from HBM).

Device pipeline per 128-column chunk:
  - center tap weight is exactly 1 (exp(0)*w1(0,0)): no weight compute,
    no multiply; its PE pass reads the src tile directly and it enters
    the denominator as the +1.0 constant folded into the Act-engine
    PSUM drain bias.
  - weights for the 8 off-center taps: subs on DVE (fp16 2x mode),
    squares batched per dy on Act (fp32), channel-sum adds batched per
    dy on Pool (fp32 is Pool's good case), per-tap exp on Act with the
    spatial ln(w1) as a per-partition bias tile.
  - products: DVE fp16 (2x) for ~6.5 taps, Pool for the rest.
  - tap accumulation on the otherwise idle PE: identity matmuls
    accumulating in PSUM, one single-bank [128,21,<=24] subslice per
    group; den likewise accumulates on the PE ([128,128] passes).
  - PSUM drains to fp16 SBUF on Act; num and den ship to HBM and the
    host does the final num/den divide (frees DVE of recip+final mult).
"""

import sys

sys.path.insert(0, "/opt/trn_rl_repo")

import numpy as np

B, H, W = 2, 512, 512
CS, CI = 21, 3
CH = CS + CI  # combined channels in the srcim slab
N_CORES = 8
ROWS = (B * H) // N_CORES  # 128 output rows per core
WC = 128  # output cols per chunk
NCH = W // WC  # 4 chunks
SLOT = WC + 2
SUB = 24  # psum subslice output columns (21*24*4B = 2016B = 1 bank)

INV2SIG2 = 8.0  # 1/(2*0.25^2)

# off-center taps (dy, dx), grouped by dy; wk9 slot = list index
TAPS = [(0, 0), (0, 1), (0, 2), (1, 0), (1, 2), (2, 0), (2, 1), (2, 2)]
DY_SLOTS = {0: (0, 3), 1: (3, 5), 2: (5, 8)}  # dy -> wk9 slot range
POOL_SPLIT = 11  # slot 6: channels [POOL_SPLIT:] go to Pool; slot 7 all Pool

_CACHE = {}


def _sub_ranges(wc):
    r = []
    x = 0
    while x < wc:
        r.append((x, min(SUB, wc - x)))
        x += SUB
    return r


def _build():
    from concourse.bacc import Bacc
    from concourse.tile import TileContext
    from concourse.masks import make_identity
    import concourse.mybir as mybir

    fp32 = mybir.dt.float32
    fp16 = mybir.dt.float16
    Alu = mybir.AluOpType
    Act = mybir.ActivationFunctionType

    nc = Bacc("TRN2", target_bir_lowering=False, debug=False, num_devices=N_CORES)
    srcim_d = nc.dram_tensor(
        "srcim", [ROWS + 2, NCH, CH, SLOT], fp16, kind="ExternalInput"
    )
    num_d = nc.dram_tensor("num", [ROWS, NCH, CS, WC], fp16, kind="ExternalOutput")
    den_d = nc.dram_tensor("den", [ROWS, NCH, WC], fp16, kind="ExternalOutput")

    with TileContext(nc) as tc:
        with tc.tile_pool(name="p", bufs=1) as pool, tc.psum_pool(
            name="ps", bufs=1
        ) as psp:
            ident = pool.tile([ROWS, ROWS], fp16, tag="ident")
            make_identity(nc, ident[:])
            bias_t = {}
            for v in (-0.5, -1.0):
                bt = pool.tile([ROWS, 1], fp32, tag=f"b{v}", name=f"b{v}")
                nc.vector.memset(bt[:], v)
                bias_t[v] = bt

            for ci in range(NCH):
                # --- three dy-shifted windows, one HBM DMA each
                g = []
                for dy in range(3):
                    gt = pool.tile(
                        [ROWS, CH, SLOT], fp16, tag=f"g{dy}", name=f"g{dy}", bufs=2
                    )
                    nc.sync.dma_start(
                        gt[:].rearrange("p c w -> p (c w)"),
                        srcim_d[dy : dy + ROWS, ci].rearrange("p c w -> p (c w)"),
                    )
                    g.append(gt)
                s_t = [gt[:, :CS] for gt in g]
                i_t = [gt[:, CS:] for gt in g]
                ic = i_t[1][:, :, 1 : 1 + WC]

                # --- bilateral weights for the 8 off-center taps
                wk9 = pool.tile([ROWS, 8, WC], fp16, tag="wk9", bufs=2)
                d3 = {}
                d23 = {}
                wr = {}
                for dy in range(3):
                    nj = DY_SLOTS[dy][1] - DY_SLOTS[dy][0]
                    d3[dy] = pool.tile(
                        [ROWS, nj, CI, WC], fp16, tag=f"d3_{dy}", name=f"d3_{dy}",
                        bufs=2,
                    )
                    d23[dy] = pool.tile(
                        [ROWS, nj, CI, WC], fp32, tag=f"d23_{dy}", name=f"d23_{dy}",
                        bufs=2,
                    )
                    wr[dy] = pool.tile(
                        [ROWS, nj, WC], fp32, tag=f"wr_{dy}", name=f"wr_{dy}", bufs=2
                    )
                for slot, (dy, dx) in enumerate(TAPS):
                    j = slot - DY_SLOTS[dy][0]
                    nc.vector.tensor_tensor(
                        d3[dy][:, j], i_t[dy][:, :, dx : dx + WC], ic, Alu.subtract
                    )
                for dy in range(3):
                    nc.scalar.square(d23[dy][:], d3[dy][:])
                    nc.gpsimd.tensor_tensor(
                        wr[dy][:], d23[dy][:, :, 0], d23[dy][:, :, 1], Alu.add
                    )
                    nc.gpsimd.tensor_tensor(
                        wr[dy][:], wr[dy][:], d23[dy][:, :, 2], Alu.add
                    )
                for slot, (dy, dx) in enumerate(TAPS):
                    j = slot - DY_SLOTS[dy][0]
                    lnw1 = -0.5 * ((dx - 1) ** 2 + (dy - 1) ** 2)
                    nc.scalar.activation(
                        wk9[:, slot],
                        wr[dy][:, j],
                        Act.Exp,
                        bias=bias_t[lnw1][:],
                        scale=-INV2SIG2,
                    )

                # --- products: DVE for slots 0-5 + slot6[:POOL_SPLIT],
                #     Pool for slot6[POOL_SPLIT:] + slot7
                prod = {}
                for slot, (dy, dx) in enumerate(TAPS):
                    pt = pool.tile(
                        [ROWS, CS, WC], fp16, tag=f"prod{slot}", name=f"prod{slot}",
                        bufs=2,
                    )
                    wk_b = (
                        wk9[:, slot]
                        .rearrange("p (o x) -> p o x", o=1)
                        .broadcast_to([ROWS, CS, WC])
                    )
                    sk = s_t[dy][:, :, dx : dx + WC]
                    if slot < 6:
                        nc.vector.tensor_tensor(pt[:], sk, wk_b, Alu.mult)
                    elif slot == 6:
                        nc.vector.tensor_tensor(
                            pt[:, :POOL_SPLIT], sk[:, :POOL_SPLIT],
                            wk_b[:, :POOL_SPLIT], Alu.mult,
                        )
                        nc.gpsimd.tensor_tensor(
                            pt[:, POOL_SPLIT:], sk[:, POOL_SPLIT:],
                            wk_b[:, POOL_SPLIT:], Alu.mult,
                        )
                    else:
                        nc.gpsimd.tensor_tensor(pt[:], sk, wk_b, Alu.mult)
                    prod[slot] = pt

                # --- den = 1 + sum(wk) on the PE; +1 rides the drain bias
                dps = psp.tile([ROWS, WC], fp32, tag="dps", bufs=2)
                for slot in range(8):
                    nc.tensor.matmul(
                        dps[:],
                        ident[:],
                        wk9[:, slot],
                        start=(slot == 0),
                        stop=(slot == 7),
                    )
                den = pool.tile([ROWS, WC], fp16, tag="den", bufs=2)
                nc.scalar.activation(den[:], dps[:], Act.Copy, bias=1.0)
                nc.scalar.dma_start(den_d[:, ci], den[:])

                # --- num accumulation on the PE, Act drains PSUM to fp16
                nsb = pool.tile([ROWS, CS, WC], fp16, tag="nsb", bufs=2)
                for sx, sw in _sub_ranges(WC):
                    ps = psp.tile(
                        [ROWS, CS, sw], fp32, tag=f"ps{sw}",
                        bufs=4 if sw == SUB else 2,
                    )
                    nc.tensor.matmul(
                        ps[:].rearrange("p c x -> p (c x)"),
                        ident[:],
                        s_t[1][:, :, 1 + sx : 1 + sx + sw],
                        start=True,
                        stop=False,
                    )
                    for slot in range(8):
                        nc.tensor.matmul(
                            ps[:].rearrange("p c x -> p (c x)"),
                            ident[:],
                            prod[slot][:, :, sx : sx + sw],
                            start=False,
                            stop=(slot == 7),
                        )
                    nc.scalar.copy(nsb[:, :, sx : sx + sw], ps[:])
                nc.scalar.dma_start(
                    num_d[:, ci].rearrange("p c w -> p (c w)"),
                    nsb[:].rearrange("p c w -> p (c w)"),
                )
    nc.compile()
    return nc


def _get_nc():
    if "nc" not in _CACHE:
        _CACHE["nc"] = _build()
    return _CACHE["nc"]


def _shard_inputs(src, im):
    srcp = np.pad(src, ((0, 0), (1, 1), (1, 1), (0, 0)), mode="reflect")
    imp = np.pad(im, ((0, 0), (1, 1), (1, 1), (0, 0)), mode="reflect")
    # channel-major fp16 combined: [B, Hp, CH, Wp]
    comb = np.concatenate(
        [np.transpose(srcp, (0, 1, 3, 2)), np.transpose(imp, (0, 1, 3, 2))], axis=2
    ).astype(np.float16)
    Hp = H + 2
    sl = np.empty((B, Hp, NCH, CH, SLOT), np.float16)
    for ci in range(NCH):
        sl[:, :, ci] = comb[:, :, :, ci * WC : ci * WC + SLOT]
    in_maps = []
    for core in range(N_CORES):
        b, r0 = core // (N_CORES // B), (core % (N_CORES // B)) * ROWS
        in_maps.append({"srcim": np.ascontiguousarray(sl[b, r0 : r0 + ROWS + 2])})
    return in_maps


def _unshard_output(results):
    out = np.empty((B, H, W, CS), dtype=np.float32)
    for core in range(N_CORES):
        b, r0 = core // (N_CORES // B), (core % (N_CORES // B)) * ROWS
        num = np.asarray(results[core]["num"], np.float32).reshape(ROWS, NCH, CS, WC)
        den = np.asarray(results[core]["den"], np.float32).reshape(ROWS, NCH, 1, WC)
        o = num / den  # [128, NCH, CS, WC]
        o = np.transpose(o, (0, 1, 3, 2)).reshape(ROWS, W, CS)
        out[b, r0 : r0 + ROWS] = o
    return out


def kernel(src, im, _trace=False, _tmpdir=None):
    from concourse import bass_utils

    src = np.asarray(src, dtype=np.float32)
    im = np.asarray(im, dtype=np.float32)
    nc = _get_nc()
    in_maps = _shard_inputs(src, im)
    res = bass_utils.run_bass_kernel_spmd(
        nc, in_maps, core_ids=list(range(N_CORES)), trace=_trace, tmpdir=_tmpdir
    )
    _CACHE["last_results"] = res
    return _unshard_output(res.results)
